# revision 1
# baseline (speedup 1.0000x reference)
"""CES layer kernel v2 for Trainium2 (8 NeuronCores, data-parallel over batch).

out[b,o] = cos(x @ angle(w).T + bias) * exp(x @ log|w|.T)

v2 design (vs v1): fp16 HBM I/O + [o, b] on-device layout.
  - host: xT per core [128, 32768] fp16 (I on partitions); wcat [I, 3O] fp16
    = [log|w|.T | angle.T/2pi hi | lo]; cvec [128, 2] fp32 = [b', c1].
  - device: W-stationary matmuls stream xT -> PSUM [o, cols]:
      mag_ps = x.L ; phi_ps = x.Th + x.Tl   (fp16, 1 cycle/col)
    ACT exp(mag_ps) -> mag16 fp16; DVE fused frac(phi+b') -> f16 fp16;
    sin2pi via ACT hw table for most groups and via a fused 8-stage DVE
    degree-7 polynomial for the rest (engine balancing); DVE fp16 4x
    multiply; straight fp16 HWDGE DMA both directions.
  - host: transpose [o,b] -> [b,o] and cast fp32.
"""

import os
import numpy as np

import concourse.bacc as bacc
import concourse.mybir as mybir
import concourse.hw_specs as hw_specs
import concourse.dve_ops as dve_ops
from concourse.tile import TileContext
from concourse.bass_utils import run_bass_kernel_spmd
from concourse.dve_spec import Spec, Src0, Src1, C0, C1, C2, lower, sq
from concourse.dve_uop import DveOpSpec

dt = mybir.dt
AF = mybir.ActivationFunctionType

B, I, O = 262144, 128, 128
N_CORES = 8
B_CORE = B // N_CORES              # 32768
# tapered super sizes: short pipeline fill/drain at the ends
SUPERS = [1024, 2048, 2048] + [4096] * 6 + [2048, 1024]
assert sum(SUPERS) == B_CORE
PAIR = 1024                        # cols per PSUM tile (2 banks x fp32)
MMW = int(os.environ.get("CES_MMW", "512"))   # matmul moving width
USE_TL = int(os.environ.get("CES_TL", "0"))   # include low-part angle matmul
DVEF = int(os.environ.get("CES_DVEF", "4"))   # 1/DVEF of sin cols on DVE poly
POOL8 = int(os.environ.get("CES_POOL8", "4"))  # POOL8/8 of mul cols on Pool
MAGIC = float(1.5 * 2 ** 23)
# minimax sin(2*pi*f) = f*(c1 + c3 s + c5 s^2 + c7 s^3), s=f^2, f in [-.5,.5]
C1S, C3S, C5S, C7S = 6.2786854, -41.09533605, 77.94322921, -56.11589314
# rescaled so the u^7 coefficient is exactly -1:  u = KS*f,
# sin(2*pi*f) = u*(D1 + D3 u^2 + D5 u^4 - u^6)
KS = float((-C7S) ** (1.0 / 7.0))
D1, D3, D5 = C1S / KS, C3S / KS ** 3, C5S / KS ** 5

_SETUP_DONE = False
ADD_FRAC_B = None
SIN_POLY7 = None


def _mk_op(name, opcode, spec, rd1):
    shas = {}
    for ver in ("v3", "v4"):
        try:
            s = DveOpSpec(name=name, opcode=opcode, uops=lower(spec, ver=ver),
                          rd1_en=rd1)
            shas[ver] = s.sha(ver)
        except Exception:
            pass
    op = dve_ops.DveOp(name, spec, subdim=False, uops_sha=shas)
    dve_ops.OPS.append(op)
    dve_ops._SUB_OPCODE_FOR_NAME[name] = opcode
    dve_ops.CUSTOM_DVE_SPECS[name] = op.spec
    return op


def _setup_framework():
    global _SETUP_DONE, ADD_FRAC_B, SIN_POLY7
    if _SETUP_DONE:
        return
    # out = frac(in0 + bias) * scale ; bias via s1 ([P,1] AP), magic via s0,
    # scale via imm2 (1.0 for the ACT sin path, KS for the DVE poly path)
    _y = Src0 + C1
    ADD_FRAC_B = _mk_op("ADD_FRAC_B", 17, Spec(
        body=(_y - ((_y + C0) - C0)) * C2,
        reference=lambda in0, in1, s0, s1, imm2:
            ((in0 + s1) - ((in0 + s1 + s0) - s0)) * imm2,
    ), rd1=False)
    # in0 = u = KS*f: out = u*(D1 + D3 u^2 + D5 u^4 - u^6) ~= sin(2*pi*f)
    # (no second tensor stream: rd1 custom ops measured ~4.5x slower)
    _u = Src0
    _s = sq(_u)
    SIN_POLY7 = _mk_op("SIN_POLY7", 18, Spec(
        body=_u * (C0 + _s * (C1 + _s * (C2 - _s))),
        reference=lambda in0, in1, s0, s1, imm2:
            in0 * (s0 + (in0 * in0) * (s1 + (in0 * in0)
                   * (imm2 - (in0 * in0)))),
    ), rd1=False)

    _real_tables = hw_specs.get_activation_tables

    def _patched_tables(arch):
        t = _real_tables(arch)
        for _name, s in t.items():
            s.discard(AF.Sin)
        t["exp_and_friends"].add(AF.Sin)
        return t

    bacc.get_activation_tables = _patched_tables
    _SETUP_DONE = True


_NC_CACHE = None


def _build_program():
    global _NC_CACHE
    if _NC_CACHE is not None:
        return _NC_CACHE
    _setup_framework()

    nc = bacc.Bacc()
    xtd = nc.dram_tensor("xt", [128, B_CORE], dt.float16, kind="ExternalInput")
    wcat_d = nc.dram_tensor("wcat", [I, 3 * O], dt.float16,
                            kind="ExternalInput")
    cvec_d = nc.dram_tensor("cvec", [128, 1], dt.float32, kind="ExternalInput")
    outd = nc.dram_tensor("out", [128, B_CORE], dt.float16,
                          kind="ExternalOutput")

    with TileContext(nc) as tc:
        with tc.tile_pool(name="const", bufs=1) as cpool, \
             tc.tile_pool(name="xb", bufs=3) as xb_pool, \
             tc.tile_pool(name="ob", bufs=3) as ob_pool, \
             tc.tile_pool(name="grp", bufs=2) as gpool, \
             tc.tile_pool(name="magps", bufs=2, space="PSUM") as mag_psum, \
             tc.tile_pool(name="phips", bufs=2, space="PSUM") as phi_psum:

            wcat = cpool.tile([I, 3 * O], dt.float16)
            nc.sync.dma_start(out=wcat, in_=wcat_d[:, :])
            cvec = cpool.tile([128, 1], dt.float32)
            nc.sync.dma_start(out=cvec, in_=cvec_d[:, :])
            bias_t = cvec[:, 0:1]

            pend = None            # (outs, mag16, sin16, col0, S, NA) of s-1

            def flush(pend):
                """Deferred multiplies + output DMA for super s-1 (its sin is
                long done, so no head-of-line blocking). Output DMA triggers
                from the gpsimd queue right after the Pool multiply, keeping
                the sync queue a pure input-prefetch stream."""
                p_outs, p_mag, p_sin, p_col0, S, NA = pend
                pc = S * POOL8 // 8
                if pc > 0:
                    nc.gpsimd.tensor_mul(p_outs[:, 0:pc], p_mag[:, 0:pc],
                                         p_sin[:, 0:pc])
                if pc < S:
                    nc.vector.tensor_mul(p_outs[:, pc:S], p_mag[:, pc:S],
                                         p_sin[:, pc:S])
                nc.gpsimd.dma_start(out=outd[:, p_col0:p_col0 + S],
                                    in_=p_outs)

            col0 = 0
            for s, S in enumerate(SUPERS):
                NA = S - S // DVEF if DVEF > 0 else S
                xs_full = xb_pool.tile([128, 4096], dt.float16, tag="xs",
                                       name="xs")
                xs = xs_full[:, 0:S]
                nc.sync.dma_start(out=xs, in_=xtd[:, col0:col0 + S])

                mag16 = gpool.tile([128, 4096], dt.float16, tag="mag16",
                                   name="mag16")[:, 0:S]
                f16 = gpool.tile([128, 4096], dt.float16, tag="f16",
                                 name="f16")[:, 0:S]
                sin16 = gpool.tile([128, 4096], dt.float16, tag="sin16",
                                   name="sin16")[:, 0:S]
                outs = ob_pool.tile([128, 4096], dt.float16, tag="outs",
                                    name="outs")[:, 0:S]

                if pend is not None:
                    flush(pend)

                sin_done = 0           # f16 cols already sent to ACT sin
                for p0 in range(0, S, PAIR):
                    PW = min(PAIR, S - p0)
                    mag_ps = mag_psum.tile([128, PAIR], dt.float32,
                                           tag="magps", name="mag_ps")[:, 0:PW]
                    phi_ps = phi_psum.tile([128, PAIR], dt.float32,
                                           tag="phips", name="phi_ps")[:, 0:PW]
                    nmm = PW // MMW
                    for h in range(nmm):
                        cc = slice(p0 + h * MMW, p0 + (h + 1) * MMW)
                        pc = slice(h * MMW, (h + 1) * MMW)
                        nc.tensor.matmul(mag_ps[:, pc], wcat[:, 0:128],
                                         xs[:, cc], start=True, stop=True)
                    for h in range(nmm):
                        cc = slice(p0 + h * MMW, p0 + (h + 1) * MMW)
                        pc = slice(h * MMW, (h + 1) * MMW)
                        nc.tensor.matmul(phi_ps[:, pc], wcat[:, 128:256],
                                         xs[:, cc], start=True,
                                         stop=(not USE_TL))
                    if USE_TL:
                        for h in range(nmm):
                            cc = slice(p0 + h * MMW, p0 + (h + 1) * MMW)
                            pc = slice(h * MMW, (h + 1) * MMW)
                            nc.tensor.matmul(phi_ps[:, pc], wcat[:, 256:384],
                                             xs[:, cc], start=False, stop=True)

                    nc.scalar.activation(mag16[:, p0:p0 + PW], mag_ps,
                                         AF.Exp, bias=0.0, scale=1.0)
                    # range-reduce; cols >= NA feed the DVE poly (scale KS)
                    lo, hi = p0, p0 + PW
                    if lo < NA:
                        e = min(hi, NA)
                        nc.vector._custom_dve(
                            ADD_FRAC_B, out=f16[:, lo:e],
                            in0=phi_ps[:, 0:e - lo], s0=MAGIC, s1=bias_t,
                            imm2=1.0)
                    if hi > NA:
                        b = max(lo, NA)
                        nc.vector._custom_dve(
                            ADD_FRAC_B, out=f16[:, b:hi],
                            in0=phi_ps[:, b - p0:PW], s0=MAGIC, s1=bias_t,
                            imm2=KS)
                    # ACT sin in ~2048-col chunks as soon as deps allow
                    avail = min(hi, NA)
                    while avail - sin_done >= 2048 or \
                            (avail == NA and avail > sin_done):
                        c1 = min(sin_done + 2048, NA)
                        nc.scalar.activation(
                            sin16[:, sin_done:c1], f16[:, sin_done:c1],
                            AF.Sin, bias=0.0, scale=1.0)
                        sin_done = c1

                # DVE poly sin
                if NA < S:
                    nc.vector._custom_dve(
                        SIN_POLY7, out=sin16[:, NA:S],
                        in0=f16[:, NA:S], s0=D1, s1=D3, imm2=D5)

                pend = (outs, mag16, sin16, col0, S, NA)
                col0 += S

            flush(pend)

    nc.compile()
    _real_tjb = nc.to_json_bytes
    nc.to_json_bytes = lambda: _real_tjb().replace(b'"func":"Sin"',
                                                   b'"func":"Sin2pi"')
    _NC_CACHE = nc
    return nc


LAST_RESULT = None


def kernel(x, w_real, w_imag, bias):
    global LAST_RESULT
    x = np.asarray(x, dtype=np.float32)
    w_real = np.asarray(w_real, dtype=np.float32)
    w_imag = np.asarray(w_imag, dtype=np.float32)
    bias = np.asarray(bias, dtype=np.float32)

    wr = w_real.astype(np.float64)
    wi = w_imag.astype(np.float64)
    L = 0.5 * np.log(wr * wr + wi * wi)            # [O, I] log|w|
    T = np.arctan2(wi, wr) / (2 * np.pi)           # [O, I] angle in turns
    Lh = np.ascontiguousarray(L.T).astype(np.float16)
    TT = np.ascontiguousarray(T.T)                 # [I, O]
    Th = TT.astype(np.float16)
    Tl = (TT - Th.astype(np.float64)).astype(np.float16)
    wcat = np.concatenate([Lh, Th, Tl], axis=1)    # [I, 3*O] fp16
    bp = ((bias.astype(np.float64) + np.pi / 2) / (2 * np.pi)).astype(
        np.float32)                                # [O] bias in turns + 1/4
    cvec = np.ascontiguousarray(bp[:, None], dtype=np.float32)  # [128, 1]

    xh = x.astype(np.float16)                      # [B, I]

    nc = _build_program()

    in_maps = []
    for c in range(N_CORES):
        xt = np.ascontiguousarray(xh[c * B_CORE:(c + 1) * B_CORE, :].T)
        in_maps.append({"xt": xt, "wcat": wcat, "cvec": cvec})

    trace = bool(int(os.environ.get("CES_TRACE", "0")))
    if trace:
        _install_trace_shim()
    try:
        res = run_bass_kernel_spmd(nc, in_maps, core_ids=list(range(N_CORES)),
                                   trace=trace)
    except Exception:
        import time
        time.sleep(2.0)
        res = run_bass_kernel_spmd(nc, in_maps, core_ids=list(range(N_CORES)),
                                   trace=False)
    LAST_RESULT = res
    if trace and res.exec_time_ns is not None:
        print(f"HW exec time: {res.exec_time_ns} ns", flush=True)

    out = np.empty((B, O), dtype=np.float32)
    for c in range(N_CORES):
        out[c * B_CORE:(c + 1) * B_CORE, :] = \
            res.results[c]["out"].T.astype(np.float32)
    return out


def _install_trace_shim():
    """NTFF profiling hook shim (this image's antenv lacks axon_hooks)."""
    import sys
    import types
    import importlib.util as ilu
    if ilu.find_spec("antenv.axon_hooks") is None and \
            "antenv.axon_hooks" not in sys.modules:
        m = types.ModuleType("antenv.axon_hooks")
        h = [None]
        m.set_axon_ntff_profile_hook = lambda v: h.__setitem__(0, v)
        m.get_axon_ntff_profile_hook = lambda: h[0]
        sys.modules["antenv.axon_hooks"] = m
        sys.path.insert(0, "/root/.axon_site")
        try:
            from trn_agent_boot.trn_boot import _ntff_profile_via_ctypes
            m.set_axon_ntff_profile_hook(
                _ntff_profile_via_ctypes("/opt/axon/libaxon_pjrt.so"))
        except Exception:
            pass
    import concourse.bass_utils as bu
    bu.upload_artifacts = lambda d: "local://skipped"


if __name__ == "__main__":
    rng = np.random.default_rng(0)
    x = rng.uniform(-1, 1, (B, I)).astype(np.float32)
    th = rng.uniform(-np.pi, np.pi, (O, I)).astype(np.float32)
    sc = np.exp(0.1 * rng.standard_normal((O, I))).astype(np.float32)
    wr = sc * np.cos(th)
    wi = sc * np.sin(th)
    bs = rng.uniform(-np.pi, np.pi, (O,)).astype(np.float32)
    got = kernel(x, wr, wi, bs)
    absw = np.sqrt(wr.astype(np.float64)**2 + wi.astype(np.float64)**2)
    angw = np.arctan2(wi.astype(np.float64), wr.astype(np.float64))
    mag = np.exp(x.astype(np.float64) @ np.log(absw).T)
    y = x.astype(np.float64) @ angw.T + bs
    ref = np.cos(y) * mag
    err = np.abs(got - ref)
    print(f"absmax={err.max():.3e} scale={np.abs(ref).max():.2f} "
          f"absmax/scale={err.max()/np.abs(ref).max():.3e} "
          f"relL2={np.linalg.norm(got-ref)/np.linalg.norm(ref):.3e}")



# revision 35
# speedup vs baseline: 1.0236x; 1.0236x over previous
"""CES layer kernel v3 for Trainium2 (8 NeuronCores, data-parallel over batch).

out[b,o] = cos(x @ angle(w).T + bias) * exp(x @ log|w|.T)

Design: fp16 HBM I/O + [o, b] on-device layout.
  - host: xT per core [128, 32768] fp16 (I on partitions); wcat [I, 3O] fp16
    = [log|w|.T | angle.T/2pi hi | lo]; cvec [128, 1] fp32 = b'.
  - device: W-stationary matmuls stream xT -> PSUM [o, cols]:
      mag_ps = x.L ; phi_ps = x.Th   (fp16, 1 col/cycle)
    ACT exp(mag_ps) -> mag16 fp16; DVE fused frac(phi+b') -> f16 fp16;
    sin2pi via ACT hw table for the first 3/4 of each super and via a
    fused 8-stage DVE degree-7 polynomial for the last 1/4 (pair-aligned,
    so ACT's last sin chunk never waits on the final pair's frac);
    final multiply split Pool (front half) / DVE 2x fp16 (back half),
    staggered so they never run concurrently (concurrent Pool+DVE muls
    collapse the DVE 2x packed mode ~4-15x); fp16 DMA both directions,
    input split in 2048-col chunks with 6-deep prefetch.
  - host: transpose [o,b] -> [b,o] and cast fp32.
Measured ~93.8us on 8 axon-tunneled cores (baseline 92.65..99.4us runs).
"""

import os
import numpy as np

import concourse.bacc as bacc
import concourse.mybir as mybir
import concourse.hw_specs as hw_specs
import concourse.dve_ops as dve_ops
from concourse.tile import TileContext
from concourse.bass_utils import run_bass_kernel_spmd
from concourse.dve_spec import Spec, Src0, Src1, C0, C1, C2, lower, sq
from concourse.dve_uop import DveOpSpec

dt = mybir.dt
AF = mybir.ActivationFunctionType

B, I, O = 262144, 128, 128
N_CORES = 8
B_CORE = B // N_CORES              # 32768
# tapered super sizes: short pipeline fill/drain at the ends
SUPERS = [1024, 2048, 2048] + [4096] * 6 + [2048, 1024]
assert sum(SUPERS) == B_CORE
PAIR = 1024                        # cols per PSUM tile (2 banks x fp32)
MMW = int(os.environ.get("CES_MMW", "512"))   # matmul moving width
USE_TL = int(os.environ.get("CES_TL", "0"))   # include low-part angle matmul
DVE64 = int(os.environ.get("CES_DVE64", "16"))  # DVE64/64 of sin cols on DVE
POOL8 = int(os.environ.get("CES_POOL8", "4"))  # POOL8/8 of mul cols on Pool
SINCH = int(os.environ.get("CES_SINCH", "2048"))  # ACT sin chunk cols
WARM = int(os.environ.get("CES_WARM", "0"))  # dummy ldweights per pair (HAM)
MAGIC = float(1.5 * 2 ** 23)
# minimax sin(2*pi*f) = f*(c1 + c3 s + c5 s^2 + c7 s^3), s=f^2, f in [-.5,.5]
C1S, C3S, C5S, C7S = 6.2786854, -41.09533605, 77.94322921, -56.11589314
# rescaled so the u^7 coefficient is exactly -1:  u = KS*f,
# sin(2*pi*f) = u*(D1 + D3 u^2 + D5 u^4 - u^6)
KS = float((-C7S) ** (1.0 / 7.0))
D1, D3, D5 = C1S / KS, C3S / KS ** 3, C5S / KS ** 5

_SETUP_DONE = False
ADD_FRAC_B = None
SIN_POLY7 = None


def _mk_op(name, opcode, spec, rd1):
    shas = {}
    for ver in ("v3", "v4"):
        try:
            s = DveOpSpec(name=name, opcode=opcode, uops=lower(spec, ver=ver),
                          rd1_en=rd1)
            shas[ver] = s.sha(ver)
        except Exception:
            pass
    op = dve_ops.DveOp(name, spec, subdim=False, uops_sha=shas)
    dve_ops.OPS.append(op)
    dve_ops._SUB_OPCODE_FOR_NAME[name] = opcode
    dve_ops.CUSTOM_DVE_SPECS[name] = op.spec
    return op


def _setup_framework():
    global _SETUP_DONE, ADD_FRAC_B, SIN_POLY7
    if _SETUP_DONE:
        return
    # out = frac(in0 + bias) * scale ; bias via s1 ([P,1] AP), magic via s0,
    # scale via imm2 (1.0 for the ACT sin path, KS for the DVE poly path)
    _y = Src0 + C1
    ADD_FRAC_B = _mk_op("ADD_FRAC_B", 17, Spec(
        body=(_y - ((_y + C0) - C0)) * C2,
        reference=lambda in0, in1, s0, s1, imm2:
            ((in0 + s1) - ((in0 + s1 + s0) - s0)) * imm2,
    ), rd1=False)
    # in0 = u = KS*f: out = u*(D1 + D3 u^2 + D5 u^4 - u^6) ~= sin(2*pi*f)
    # (no second tensor stream: rd1 custom ops measured ~4.5x slower)
    _u = Src0
    _s = sq(_u)
    SIN_POLY7 = _mk_op("SIN_POLY7", 18, Spec(
        body=_u * (C0 + _s * (C1 + _s * (C2 - _s))),
        reference=lambda in0, in1, s0, s1, imm2:
            in0 * (s0 + (in0 * in0) * (s1 + (in0 * in0)
                   * (imm2 - (in0 * in0)))),
    ), rd1=False)

    _real_tables = hw_specs.get_activation_tables

    def _patched_tables(arch):
        t = _real_tables(arch)
        for _name, s in t.items():
            s.discard(AF.Sin)
        t["exp_and_friends"].add(AF.Sin)
        return t

    bacc.get_activation_tables = _patched_tables
    _SETUP_DONE = True


_NC_CACHE = None


def _build_program():
    global _NC_CACHE
    if _NC_CACHE is not None:
        return _NC_CACHE
    _setup_framework()

    nc = bacc.Bacc()
    xtd = nc.dram_tensor("xt", [128, B_CORE], dt.float16, kind="ExternalInput")
    wcat_d = nc.dram_tensor("wcat", [I, 3 * O], dt.float16,
                            kind="ExternalInput")
    cvec_d = nc.dram_tensor("cvec", [128, 1], dt.float32, kind="ExternalInput")
    outd = nc.dram_tensor("out", [128, B_CORE], dt.float16,
                          kind="ExternalOutput")

    with TileContext(nc) as tc:
        with tc.tile_pool(name="const", bufs=1) as cpool, \
             tc.tile_pool(name="xb", bufs=6) as xb_pool, \
             tc.tile_pool(name="ob", bufs=4) as ob_pool, \
             tc.tile_pool(name="grp", bufs=3) as gpool, \
             tc.tile_pool(name="magps", bufs=2, space="PSUM") as mag_psum, \
             tc.tile_pool(name="phips", bufs=(3 if WARM else 2),
                          space="PSUM") as phi_psum, \
             tc.tile_pool(name="warmps", bufs=1, space="PSUM") as warm_psum:

            # scratch PSUM bank for dep-free PE-warming dummy matmuls (the
            # HAM clock gate re-throttles the PE to 1.2GHz when its duty
            # cycle drops; see engines/01-tensor-engine.md)
            warm_ps = warm_psum.tile([128, MMW], dt.float32,
                                     name="warm_ps") if WARM else None

            wcat = cpool.tile([I, 3 * O], dt.float16)
            nc.sync.dma_start(out=wcat, in_=wcat_d[:, :])
            cvec = cpool.tile([128, 1], dt.float32)
            nc.sync.dma_start(out=cvec, in_=cvec_d[:, :])
            bias_t = cvec[:, 0:1]

            pend = None    # (outs, mag16, sin16, col0, S, NA, pool8) of s-1

            def flush_pool(pend):
                """Pool multiply for super s-1 on the FRONT columns (their
                sin chunk lands earliest), issued at the start of super s's
                emission: it runs while DVE chews on s's fracs, so it never
                overlaps the DVE multiply (concurrent Pool+DVE muls collapse
                the DVE 2x packed mode ~4-15x)."""
                p_outs, p_mag, p_sin, p_col0, S, NA, pl8 = pend
                pc = S * pl8 // 8
                if pc > 0:
                    nc.gpsimd.tensor_mul(p_outs[:, 0:pc], p_mag[:, 0:pc],
                                         p_sin[:, 0:pc])
                    # ship the Pool-multiplied front half immediately; the
                    # back half goes out with flush_vec_dma
                    nc.gpsimd.dma_start(out=outd[:, p_col0:p_col0 + pc],
                                        in_=p_outs[:, 0:pc])

            def flush_vec_dma(pend):
                """DVE multiply for super s-1 on the BACK columns (emitted
                mid-super-s, after the frac pairs — by then s-1's last sin
                chunk and poly are long done and the Pool mul has finished,
                so the binding DVE engine never idles on it) + the output
                DMA, emitted after BOTH muls so the Tile dep tracker orders
                the DMA read behind the DVE writes."""
                p_outs, p_mag, p_sin, p_col0, S, NA, pl8 = pend
                pc = S * pl8 // 8
                if pc < S:
                    nc.vector.tensor_mul(p_outs[:, pc:S], p_mag[:, pc:S],
                                         p_sin[:, pc:S])
                nc.gpsimd.dma_start(out=outd[:, p_col0 + pc:p_col0 + S],
                                    in_=p_outs[:, pc:S])

            col0 = 0
            for s, S in enumerate(SUPERS):
                NA = S - (S * DVE64) // 64
                xs_full = xb_pool.tile([128, 4096], dt.float16, tag="xs",
                                       name="xs")
                xs = xs_full[:, 0:S]
                # split the input DMA so the first pairs' matmuls can start
                # before the whole super has landed (Tile deps are
                # range-aware)
                for d0 in range(0, S, 2048):
                    d1 = min(d0 + 2048, S)
                    nc.sync.dma_start(out=xs[:, d0:d1],
                                      in_=xtd[:, col0 + d0:col0 + d1])

                mag16 = gpool.tile([128, 4096], dt.float16, tag="mag16",
                                   name="mag16")[:, 0:S]
                f16 = gpool.tile([128, 4096], dt.float16, tag="f16",
                                 name="f16")[:, 0:S]
                sin16 = gpool.tile([128, 4096], dt.float16, tag="sin16",
                                   name="sin16")[:, 0:S]
                outs = ob_pool.tile([128, 4096], dt.float16, tag="outs",
                                    name="outs")[:, 0:S]

                if pend is not None:
                    flush_pool(pend)

                sin_done = 0           # f16 cols already sent to ACT sin
                for p0 in range(0, S, PAIR):
                    PW = min(PAIR, S - p0)
                    mag_ps = mag_psum.tile([128, PAIR], dt.float32,
                                           tag="magps", name="mag_ps")[:, 0:PW]
                    nmm = PW // MMW
                    if WARM:
                        phis = [(p0 + h * MMW, p0 + (h + 1) * MMW,
                                 phi_psum.tile([128, MMW], dt.float32,
                                               tag="phips", name="phi_ps"))
                                for h in range(nmm)]
                    else:
                        phi_ps = phi_psum.tile([128, PAIR], dt.float32,
                                               tag="phips",
                                               name="phi_ps")[:, 0:PW]
                        phis = [(p0, p0 + PW, phi_ps)]
                    for h in range(nmm):
                        cc = slice(p0 + h * MMW, p0 + (h + 1) * MMW)
                        pc = slice(h * MMW, (h + 1) * MMW)
                        nc.tensor.matmul(mag_ps[:, pc], wcat[:, 0:128],
                                         xs[:, cc], start=True, stop=True)
                    for lo, hi, ph in phis:
                        for h in range((hi - lo) // MMW):
                            cc = slice(lo + h * MMW, lo + (h + 1) * MMW)
                            pc = slice(h * MMW, (h + 1) * MMW)
                            nc.tensor.matmul(ph[:, pc], wcat[:, 128:256],
                                             xs[:, cc], start=True,
                                             stop=(not USE_TL))
                            if USE_TL:
                                nc.tensor.matmul(ph[:, pc],
                                                 wcat[:, 256:384],
                                                 xs[:, cc], start=False,
                                                 stop=True)
                    # dep-free PE filler: keeps the HAM activity monitor from
                    # re-throttling the PE clock to 1.2GHz during idle gaps
                    for _ in range(WARM):
                        nc.tensor.matmul(warm_ps[:, 0:384], wcat[:, 0:128],
                                         wcat[:, 0:384], start=True,
                                         stop=True)

                    nc.scalar.activation(mag16[:, p0:p0 + PW], mag_ps,
                                         AF.Exp, bias=0.0, scale=1.0)
                    # range-reduce; cols >= NA feed the DVE poly (scale KS)
                    for lo, hi, ph in phis:
                        if lo < NA:
                            e = min(hi, NA)
                            nc.vector._custom_dve(
                                ADD_FRAC_B, out=f16[:, lo:e],
                                in0=ph[:, 0:e - lo], s0=MAGIC, s1=bias_t,
                                imm2=1.0)
                        if hi > NA:
                            b = max(lo, NA)
                            nc.vector._custom_dve(
                                ADD_FRAC_B, out=f16[:, b:hi],
                                in0=ph[:, b - lo:hi - lo], s0=MAGIC,
                                s1=bias_t, imm2=KS)
                    # ACT sin in SINCH-col chunks as soon as deps allow
                    avail = min(p0 + PW, NA)
                    while avail - sin_done >= SINCH or \
                            (avail >= NA and avail > sin_done):
                        c1 = min(sin_done + SINCH, NA)
                        nc.scalar.activation(
                            sin16[:, sin_done:c1], f16[:, sin_done:c1],
                            AF.Sin, bias=0.0, scale=1.0)
                        sin_done = c1

                if pend is not None:
                    flush_vec_dma(pend)

                # DVE poly sin
                if NA < S:
                    nc.vector._custom_dve(
                        SIN_POLY7, out=sin16[:, NA:S],
                        in0=f16[:, NA:S], s0=D1, s1=D3, imm2=D5)

                pend = (outs, mag16, sin16, col0, S, NA,
                        POOL8 if s < len(SUPERS) - 1 else 0)
                col0 += S

            flush_pool(pend)
            flush_vec_dma(pend)

    nc.compile()
    _real_tjb = nc.to_json_bytes
    nc.to_json_bytes = lambda: _real_tjb().replace(b'"func":"Sin"',
                                                   b'"func":"Sin2pi"')
    _NC_CACHE = nc
    return nc


LAST_RESULT = None


def kernel(x, w_real, w_imag, bias):
    global LAST_RESULT
    x = np.asarray(x, dtype=np.float32)
    w_real = np.asarray(w_real, dtype=np.float32)
    w_imag = np.asarray(w_imag, dtype=np.float32)
    bias = np.asarray(bias, dtype=np.float32)

    wr = w_real.astype(np.float64)
    wi = w_imag.astype(np.float64)
    L = 0.5 * np.log(wr * wr + wi * wi)            # [O, I] log|w|
    T = np.arctan2(wi, wr) / (2 * np.pi)           # [O, I] angle in turns
    Lh = np.ascontiguousarray(L.T).astype(np.float16)
    TT = np.ascontiguousarray(T.T)                 # [I, O]
    Th = TT.astype(np.float16)
    Tl = (TT - Th.astype(np.float64)).astype(np.float16)
    wcat = np.concatenate([Lh, Th, Tl], axis=1)    # [I, 3*O] fp16
    bp = ((bias.astype(np.float64) + np.pi / 2) / (2 * np.pi)).astype(
        np.float32)                                # [O] bias in turns + 1/4
    cvec = np.ascontiguousarray(bp[:, None], dtype=np.float32)  # [128, 1]

    xh = x.astype(np.float16)                      # [B, I]

    nc = _build_program()

    in_maps = []
    for c in range(N_CORES):
        xt = np.ascontiguousarray(xh[c * B_CORE:(c + 1) * B_CORE, :].T)
        in_maps.append({"xt": xt, "wcat": wcat, "cvec": cvec})

    trace = bool(int(os.environ.get("CES_TRACE", "0")))
    if trace:
        _install_trace_shim()
    try:
        res = run_bass_kernel_spmd(nc, in_maps, core_ids=list(range(N_CORES)),
                                   trace=trace)
    except Exception:
        import time
        time.sleep(2.0)
        res = run_bass_kernel_spmd(nc, in_maps, core_ids=list(range(N_CORES)),
                                   trace=False)
    LAST_RESULT = res
    if trace and res.exec_time_ns is not None:
        print(f"HW exec time: {res.exec_time_ns} ns", flush=True)

    out = np.empty((B, O), dtype=np.float32)
    for c in range(N_CORES):
        out[c * B_CORE:(c + 1) * B_CORE, :] = \
            res.results[c]["out"].T.astype(np.float32)
    return out


def _install_trace_shim():
    """NTFF profiling hook shim (this image's antenv lacks axon_hooks)."""
    import sys
    import types
    import importlib.util as ilu
    if ilu.find_spec("antenv.axon_hooks") is None and \
            "antenv.axon_hooks" not in sys.modules:
        m = types.ModuleType("antenv.axon_hooks")
        h = [None]
        m.set_axon_ntff_profile_hook = lambda v: h.__setitem__(0, v)
        m.get_axon_ntff_profile_hook = lambda: h[0]
        sys.modules["antenv.axon_hooks"] = m
        sys.path.insert(0, "/root/.axon_site")
        try:
            from trn_agent_boot.trn_boot import _ntff_profile_via_ctypes
            m.set_axon_ntff_profile_hook(
                _ntff_profile_via_ctypes("/opt/axon/libaxon_pjrt.so"))
        except Exception:
            pass
    import concourse.bass_utils as bu
    bu.upload_artifacts = lambda d: "local://skipped"


if __name__ == "__main__":
    rng = np.random.default_rng(0)
    x = rng.uniform(-1, 1, (B, I)).astype(np.float32)
    th = rng.uniform(-np.pi, np.pi, (O, I)).astype(np.float32)
    sc = np.exp(0.1 * rng.standard_normal((O, I))).astype(np.float32)
    wr = sc * np.cos(th)
    wi = sc * np.sin(th)
    bs = rng.uniform(-np.pi, np.pi, (O,)).astype(np.float32)
    got = kernel(x, wr, wi, bs)
    absw = np.sqrt(wr.astype(np.float64)**2 + wi.astype(np.float64)**2)
    angw = np.arctan2(wi.astype(np.float64), wr.astype(np.float64))
    mag = np.exp(x.astype(np.float64) @ np.log(absw).T)
    y = x.astype(np.float64) @ angw.T + bs
    ref = np.cos(y) * mag
    err = np.abs(got - ref)
    print(f"absmax={err.max():.3e} scale={np.abs(ref).max():.2f} "
          f"absmax/scale={err.max()/np.abs(ref).max():.3e} "
          f"relL2={np.linalg.norm(got-ref)/np.linalg.norm(ref):.3e}")



# revision 36
# speedup vs baseline: 1.0387x; 1.0147x over previous
"""CES layer kernel v3 for Trainium2 (8 NeuronCores, data-parallel over batch).

out[b,o] = cos(x @ angle(w).T + bias) * exp(x @ log|w|.T)

Design: fp16 HBM I/O + [o, b] on-device layout.
  - host: xT per core [128, 32768] fp16 (I on partitions); wcat [I, 3O] fp16
    = [log|w|.T | angle.T/2pi hi | lo]; cvec [128, 1] fp32 = b'.
  - device: W-stationary matmuls stream xT -> PSUM [o, cols]:
      mag_ps = x.L ; phi_ps = x.Th   (fp16, 1 col/cycle)
    ACT exp(mag_ps) -> mag16 fp16; DVE fused frac(phi+b') -> f16 fp16;
    sin2pi via ACT hw table for the first 3/4 of each super and via a
    fused 8-stage DVE degree-7 polynomial for the last 1/4 (pair-aligned,
    so ACT's last sin chunk never waits on the final pair's frac);
    final multiply split Pool (front half) / DVE 2x fp16 (back half),
    staggered so they never run concurrently (concurrent Pool+DVE muls
    collapse the DVE 2x packed mode ~4-15x); fp16 DMA both directions,
    input split in 2048-col chunks with 6-deep prefetch.
  - host: transpose [o,b] -> [b,o] and cast fp32.
Measured 92.7-95.1us across runs on 8 axon-tunneled cores (rested device;
back-to-back runs can read ~110us from chip-level DVFS throttling).
Baseline v2 measured 95.9-99.4us under the same conditions.
"""

import os
import numpy as np

import concourse.bacc as bacc
import concourse.mybir as mybir
import concourse.hw_specs as hw_specs
import concourse.dve_ops as dve_ops
from concourse.tile import TileContext
from concourse.bass_utils import run_bass_kernel_spmd
from concourse.dve_spec import Spec, Src0, Src1, C0, C1, C2, lower, sq
from concourse.dve_uop import DveOpSpec

dt = mybir.dt
AF = mybir.ActivationFunctionType

B, I, O = 262144, 128, 128
N_CORES = 8
B_CORE = B // N_CORES              # 32768
# tapered super sizes: short pipeline fill/drain at the ends
SUPERS = [1024, 2048, 2048] + [4096] * 6 + [2048, 1024]
assert sum(SUPERS) == B_CORE
PAIR = 1024                        # cols per PSUM tile (2 banks x fp32)
MMW = int(os.environ.get("CES_MMW", "512"))   # matmul moving width
USE_TL = int(os.environ.get("CES_TL", "0"))   # include low-part angle matmul
DVE64 = int(os.environ.get("CES_DVE64", "16"))  # DVE64/64 of sin cols on DVE
POOL8 = int(os.environ.get("CES_POOL8", "4"))  # POOL8/8 of mul cols on Pool
SINCH = int(os.environ.get("CES_SINCH", "2048"))  # ACT sin chunk cols
WARM = int(os.environ.get("CES_WARM", "0"))  # dummy ldweights per pair (HAM)
MAGIC = float(1.5 * 2 ** 23)
# minimax sin(2*pi*f) = f*(c1 + c3 s + c5 s^2 + c7 s^3), s=f^2, f in [-.5,.5]
C1S, C3S, C5S, C7S = 6.2786854, -41.09533605, 77.94322921, -56.11589314
# rescaled so the u^7 coefficient is exactly -1:  u = KS*f,
# sin(2*pi*f) = u*(D1 + D3 u^2 + D5 u^4 - u^6)
KS = float((-C7S) ** (1.0 / 7.0))
D1, D3, D5 = C1S / KS, C3S / KS ** 3, C5S / KS ** 5

_SETUP_DONE = False
ADD_FRAC_B = None
SIN_POLY7 = None


def _mk_op(name, opcode, spec, rd1):
    shas = {}
    for ver in ("v3", "v4"):
        try:
            s = DveOpSpec(name=name, opcode=opcode, uops=lower(spec, ver=ver),
                          rd1_en=rd1)
            shas[ver] = s.sha(ver)
        except Exception:
            pass
    op = dve_ops.DveOp(name, spec, subdim=False, uops_sha=shas)
    dve_ops.OPS.append(op)
    dve_ops._SUB_OPCODE_FOR_NAME[name] = opcode
    dve_ops.CUSTOM_DVE_SPECS[name] = op.spec
    return op


def _setup_framework():
    global _SETUP_DONE, ADD_FRAC_B, SIN_POLY7
    if _SETUP_DONE:
        return
    # out = frac(in0 + bias) * scale ; bias via s1 ([P,1] AP), magic via s0,
    # scale via imm2 (1.0 for the ACT sin path, KS for the DVE poly path)
    _y = Src0 + C1
    ADD_FRAC_B = _mk_op("ADD_FRAC_B", 17, Spec(
        body=(_y - ((_y + C0) - C0)) * C2,
        reference=lambda in0, in1, s0, s1, imm2:
            ((in0 + s1) - ((in0 + s1 + s0) - s0)) * imm2,
    ), rd1=False)
    # in0 = u = KS*f: out = u*(D1 + D3 u^2 + D5 u^4 - u^6) ~= sin(2*pi*f)
    # (no second tensor stream: rd1 custom ops measured ~4.5x slower)
    _u = Src0
    _s = sq(_u)
    SIN_POLY7 = _mk_op("SIN_POLY7", 18, Spec(
        body=_u * (C0 + _s * (C1 + _s * (C2 - _s))),
        reference=lambda in0, in1, s0, s1, imm2:
            in0 * (s0 + (in0 * in0) * (s1 + (in0 * in0)
                   * (imm2 - (in0 * in0)))),
    ), rd1=False)

    _real_tables = hw_specs.get_activation_tables

    def _patched_tables(arch):
        t = _real_tables(arch)
        for _name, s in t.items():
            s.discard(AF.Sin)
        t["exp_and_friends"].add(AF.Sin)
        return t

    bacc.get_activation_tables = _patched_tables
    _SETUP_DONE = True


_NC_CACHE = None


def _build_program():
    global _NC_CACHE
    if _NC_CACHE is not None:
        return _NC_CACHE
    _setup_framework()

    nc = bacc.Bacc()
    xtd = nc.dram_tensor("xt", [128, B_CORE], dt.float16, kind="ExternalInput")
    wcat_d = nc.dram_tensor("wcat", [I, 3 * O], dt.float16,
                            kind="ExternalInput")
    cvec_d = nc.dram_tensor("cvec", [128, 1], dt.float32, kind="ExternalInput")
    outd = nc.dram_tensor("out", [128, B_CORE], dt.float16,
                          kind="ExternalOutput")

    with TileContext(nc) as tc:
        with tc.tile_pool(name="const", bufs=1) as cpool, \
             tc.tile_pool(name="xb", bufs=6) as xb_pool, \
             tc.tile_pool(name="ob", bufs=4) as ob_pool, \
             tc.tile_pool(name="grp", bufs=3) as gpool, \
             tc.tile_pool(name="magps", bufs=2, space="PSUM") as mag_psum, \
             tc.tile_pool(name="phips", bufs=(3 if WARM else 2),
                          space="PSUM") as phi_psum, \
             tc.tile_pool(name="warmps", bufs=1, space="PSUM") as warm_psum:

            # scratch PSUM bank for dep-free PE-warming dummy matmuls (the
            # HAM clock gate re-throttles the PE to 1.2GHz when its duty
            # cycle drops; see engines/01-tensor-engine.md)
            warm_ps = warm_psum.tile([128, MMW], dt.float32,
                                     name="warm_ps") if WARM else None

            wcat = cpool.tile([I, 3 * O], dt.float16)
            nc.sync.dma_start(out=wcat, in_=wcat_d[:, :])
            cvec = cpool.tile([128, 1], dt.float32)
            nc.sync.dma_start(out=cvec, in_=cvec_d[:, :])
            bias_t = cvec[:, 0:1]

            pend = None    # (outs, mag16, sin16, col0, S, NA, pool8) of s-1

            def flush_pool(pend):
                """Pool multiply for super s-1 on the FRONT columns (their
                sin chunk lands earliest), issued at the start of super s's
                emission: it runs while DVE chews on s's fracs, so it never
                overlaps the DVE multiply (concurrent Pool+DVE muls collapse
                the DVE 2x packed mode ~4-15x)."""
                p_outs, p_mag, p_sin, p_col0, S, NA, pl8 = pend
                pc = S * pl8 // 8
                if pc > 0:
                    nc.gpsimd.tensor_mul(p_outs[:, 0:pc], p_mag[:, 0:pc],
                                         p_sin[:, 0:pc])
                    # ship the Pool-multiplied front half immediately; the
                    # back half goes out with flush_vec_dma
                    nc.gpsimd.dma_start(out=outd[:, p_col0:p_col0 + pc],
                                        in_=p_outs[:, 0:pc])

            def flush_vec_dma(pend):
                """DVE multiply for super s-1 on the BACK columns (emitted
                mid-super-s, after the frac pairs — by then s-1's last sin
                chunk and poly are long done and the Pool mul has finished,
                so the binding DVE engine never idles on it) + the output
                DMA, emitted after BOTH muls so the Tile dep tracker orders
                the DMA read behind the DVE writes."""
                p_outs, p_mag, p_sin, p_col0, S, NA, pl8 = pend
                pc = S * pl8 // 8
                if pc < S:
                    nc.vector.tensor_mul(p_outs[:, pc:S], p_mag[:, pc:S],
                                         p_sin[:, pc:S])
                nc.gpsimd.dma_start(out=outd[:, p_col0 + pc:p_col0 + S],
                                    in_=p_outs[:, pc:S])

            col0 = 0
            for s, S in enumerate(SUPERS):
                NA = S - (S * DVE64) // 64
                xs_full = xb_pool.tile([128, 4096], dt.float16, tag="xs",
                                       name="xs")
                xs = xs_full[:, 0:S]
                # split the input DMA so the first pairs' matmuls can start
                # before the whole super has landed (Tile deps are
                # range-aware)
                for d0 in range(0, S, 2048):
                    d1 = min(d0 + 2048, S)
                    nc.sync.dma_start(out=xs[:, d0:d1],
                                      in_=xtd[:, col0 + d0:col0 + d1])

                mag16 = gpool.tile([128, 4096], dt.float16, tag="mag16",
                                   name="mag16")[:, 0:S]
                f16 = gpool.tile([128, 4096], dt.float16, tag="f16",
                                 name="f16")[:, 0:S]
                sin16 = gpool.tile([128, 4096], dt.float16, tag="sin16",
                                   name="sin16")[:, 0:S]
                outs = ob_pool.tile([128, 4096], dt.float16, tag="outs",
                                    name="outs")[:, 0:S]

                if pend is not None:
                    flush_pool(pend)

                sin_done = 0           # f16 cols already sent to ACT sin
                for p0 in range(0, S, PAIR):
                    PW = min(PAIR, S - p0)
                    mag_ps = mag_psum.tile([128, PAIR], dt.float32,
                                           tag="magps", name="mag_ps")[:, 0:PW]
                    nmm = PW // MMW
                    if WARM:
                        phis = [(p0 + h * MMW, p0 + (h + 1) * MMW,
                                 phi_psum.tile([128, MMW], dt.float32,
                                               tag="phips", name="phi_ps"))
                                for h in range(nmm)]
                    else:
                        phi_ps = phi_psum.tile([128, PAIR], dt.float32,
                                               tag="phips",
                                               name="phi_ps")[:, 0:PW]
                        phis = [(p0, p0 + PW, phi_ps)]
                    for h in range(nmm):
                        cc = slice(p0 + h * MMW, p0 + (h + 1) * MMW)
                        pc = slice(h * MMW, (h + 1) * MMW)
                        nc.tensor.matmul(mag_ps[:, pc], wcat[:, 0:128],
                                         xs[:, cc], start=True, stop=True)
                    for lo, hi, ph in phis:
                        for h in range((hi - lo) // MMW):
                            cc = slice(lo + h * MMW, lo + (h + 1) * MMW)
                            pc = slice(h * MMW, (h + 1) * MMW)
                            nc.tensor.matmul(ph[:, pc], wcat[:, 128:256],
                                             xs[:, cc], start=True,
                                             stop=(not USE_TL))
                            if USE_TL:
                                nc.tensor.matmul(ph[:, pc],
                                                 wcat[:, 256:384],
                                                 xs[:, cc], start=False,
                                                 stop=True)
                    # dep-free PE filler: keeps the HAM activity monitor from
                    # re-throttling the PE clock to 1.2GHz during idle gaps
                    for _ in range(WARM):
                        nc.tensor.matmul(warm_ps[:, 0:384], wcat[:, 0:128],
                                         wcat[:, 0:384], start=True,
                                         stop=True)

                    nc.scalar.activation(mag16[:, p0:p0 + PW], mag_ps,
                                         AF.Exp, bias=0.0, scale=1.0)
                    # range-reduce; cols >= NA feed the DVE poly (scale KS)
                    for lo, hi, ph in phis:
                        if lo < NA:
                            e = min(hi, NA)
                            nc.vector._custom_dve(
                                ADD_FRAC_B, out=f16[:, lo:e],
                                in0=ph[:, 0:e - lo], s0=MAGIC, s1=bias_t,
                                imm2=1.0)
                        if hi > NA:
                            b = max(lo, NA)
                            nc.vector._custom_dve(
                                ADD_FRAC_B, out=f16[:, b:hi],
                                in0=ph[:, b - lo:hi - lo], s0=MAGIC,
                                s1=bias_t, imm2=KS)
                    # ACT sin in SINCH-col chunks as soon as deps allow
                    avail = min(p0 + PW, NA)
                    while avail - sin_done >= SINCH or \
                            (avail >= NA and avail > sin_done):
                        c1 = min(sin_done + SINCH, NA)
                        nc.scalar.activation(
                            sin16[:, sin_done:c1], f16[:, sin_done:c1],
                            AF.Sin, bias=0.0, scale=1.0)
                        sin_done = c1

                if pend is not None:
                    flush_vec_dma(pend)

                # DVE poly sin
                if NA < S:
                    nc.vector._custom_dve(
                        SIN_POLY7, out=sin16[:, NA:S],
                        in0=f16[:, NA:S], s0=D1, s1=D3, imm2=D5)

                pend = (outs, mag16, sin16, col0, S, NA,
                        POOL8 if s < len(SUPERS) - 1 else 0)
                col0 += S

            flush_pool(pend)
            flush_vec_dma(pend)

    nc.compile()
    _real_tjb = nc.to_json_bytes
    nc.to_json_bytes = lambda: _real_tjb().replace(b'"func":"Sin"',
                                                   b'"func":"Sin2pi"')
    _NC_CACHE = nc
    return nc


LAST_RESULT = None


def kernel(x, w_real, w_imag, bias):
    global LAST_RESULT
    x = np.asarray(x, dtype=np.float32)
    w_real = np.asarray(w_real, dtype=np.float32)
    w_imag = np.asarray(w_imag, dtype=np.float32)
    bias = np.asarray(bias, dtype=np.float32)

    wr = w_real.astype(np.float64)
    wi = w_imag.astype(np.float64)
    L = 0.5 * np.log(wr * wr + wi * wi)            # [O, I] log|w|
    T = np.arctan2(wi, wr) / (2 * np.pi)           # [O, I] angle in turns
    Lh = np.ascontiguousarray(L.T).astype(np.float16)
    TT = np.ascontiguousarray(T.T)                 # [I, O]
    Th = TT.astype(np.float16)
    Tl = (TT - Th.astype(np.float64)).astype(np.float16)
    wcat = np.concatenate([Lh, Th, Tl], axis=1)    # [I, 3*O] fp16
    bp = ((bias.astype(np.float64) + np.pi / 2) / (2 * np.pi)).astype(
        np.float32)                                # [O] bias in turns + 1/4
    cvec = np.ascontiguousarray(bp[:, None], dtype=np.float32)  # [128, 1]

    xh = x.astype(np.float16)                      # [B, I]

    nc = _build_program()

    in_maps = []
    for c in range(N_CORES):
        xt = np.ascontiguousarray(xh[c * B_CORE:(c + 1) * B_CORE, :].T)
        in_maps.append({"xt": xt, "wcat": wcat, "cvec": cvec})

    trace = bool(int(os.environ.get("CES_TRACE", "0")))
    if trace:
        _install_trace_shim()
    try:
        res = run_bass_kernel_spmd(nc, in_maps, core_ids=list(range(N_CORES)),
                                   trace=trace)
    except Exception:
        import time
        time.sleep(2.0)
        res = run_bass_kernel_spmd(nc, in_maps, core_ids=list(range(N_CORES)),
                                   trace=False)
    LAST_RESULT = res
    if trace and res.exec_time_ns is not None:
        print(f"HW exec time: {res.exec_time_ns} ns", flush=True)

    out = np.empty((B, O), dtype=np.float32)
    for c in range(N_CORES):
        out[c * B_CORE:(c + 1) * B_CORE, :] = \
            res.results[c]["out"].T.astype(np.float32)
    return out


def _install_trace_shim():
    """NTFF profiling hook shim (this image's antenv lacks axon_hooks)."""
    import sys
    import types
    import importlib.util as ilu
    if ilu.find_spec("antenv.axon_hooks") is None and \
            "antenv.axon_hooks" not in sys.modules:
        m = types.ModuleType("antenv.axon_hooks")
        h = [None]
        m.set_axon_ntff_profile_hook = lambda v: h.__setitem__(0, v)
        m.get_axon_ntff_profile_hook = lambda: h[0]
        sys.modules["antenv.axon_hooks"] = m
        sys.path.insert(0, "/root/.axon_site")
        try:
            from trn_agent_boot.trn_boot import _ntff_profile_via_ctypes
            m.set_axon_ntff_profile_hook(
                _ntff_profile_via_ctypes("/opt/axon/libaxon_pjrt.so"))
        except Exception:
            pass
    import concourse.bass_utils as bu
    bu.upload_artifacts = lambda d: "local://skipped"


if __name__ == "__main__":
    rng = np.random.default_rng(0)
    x = rng.uniform(-1, 1, (B, I)).astype(np.float32)
    th = rng.uniform(-np.pi, np.pi, (O, I)).astype(np.float32)
    sc = np.exp(0.1 * rng.standard_normal((O, I))).astype(np.float32)
    wr = sc * np.cos(th)
    wi = sc * np.sin(th)
    bs = rng.uniform(-np.pi, np.pi, (O,)).astype(np.float32)
    got = kernel(x, wr, wi, bs)
    absw = np.sqrt(wr.astype(np.float64)**2 + wi.astype(np.float64)**2)
    angw = np.arctan2(wi.astype(np.float64), wr.astype(np.float64))
    mag = np.exp(x.astype(np.float64) @ np.log(absw).T)
    y = x.astype(np.float64) @ angw.T + bs
    ref = np.cos(y) * mag
    err = np.abs(got - ref)
    print(f"absmax={err.max():.3e} scale={np.abs(ref).max():.2f} "
          f"absmax/scale={err.max()/np.abs(ref).max():.3e} "
          f"relL2={np.linalg.norm(got-ref)/np.linalg.norm(ref):.3e}")



# revision 37
# speedup vs baseline: 1.0719x; 1.0321x over previous
"""CES layer kernel v3 for Trainium2 (8 NeuronCores, data-parallel over batch).

out[b,o] = cos(x @ angle(w).T + bias) * exp(x @ log|w|.T)

Design: fp16 HBM I/O + [o, b] on-device layout.
  - host: xT per core [128, 32768] fp16 (I on partitions); wcat [I, 3O] fp16
    = [log|w|.T | angle.T/2pi hi | lo]; cvec [128, 1] fp32 = b'.
  - device: W-stationary matmuls stream xT -> PSUM [o, cols]:
      mag_ps = x.L ; phi_ps = x.Th   (fp16, 1 col/cycle)
    ACT exp(mag_ps) -> mag16 fp16; DVE fused frac(phi+b') -> f16 fp16;
    sin2pi via ACT hw table for the first 3/4 of each super and via a
    fused 8-stage DVE degree-7 polynomial for the last 1/4 (pair-aligned,
    so ACT's last sin chunk never waits on the final pair's frac);
    final multiply split Pool (front half) / DVE 2x fp16 (back half),
    staggered so they never run concurrently (concurrent Pool+DVE muls
    collapse the DVE 2x packed mode ~4-15x); fp16 DMA both directions,
    input split in 2048-col chunks with 6-deep prefetch.
  - host: transpose [o,b] -> [b,o] and cast fp32.
Measured 92.7-95.1us across runs on 8 axon-tunneled cores (rested device;
back-to-back runs can read ~110us from chip-level DVFS throttling).
Baseline v2 measured 95.9-99.4us under the same conditions.
"""

import os
import numpy as np

import concourse.bacc as bacc
import concourse.mybir as mybir
import concourse.hw_specs as hw_specs
import concourse.dve_ops as dve_ops
from concourse.tile import TileContext
from concourse.bass_utils import run_bass_kernel_spmd
from concourse.dve_spec import Spec, Src0, Src1, C0, C1, C2, lower, sq
from concourse.dve_uop import DveOpSpec

dt = mybir.dt
AF = mybir.ActivationFunctionType

B, I, O = 262144, 128, 128
N_CORES = 8
B_CORE = B // N_CORES              # 32768
# tapered super sizes: short pipeline fill/drain at the ends
SUPERS = [1024, 2048, 2048] + [4096] * 6 + [2048, 1024]
assert sum(SUPERS) == B_CORE
PAIR = 1024                        # cols per PSUM tile (2 banks x fp32)
MMW = int(os.environ.get("CES_MMW", "512"))   # matmul moving width
USE_TL = int(os.environ.get("CES_TL", "0"))   # include low-part angle matmul
DVE64 = int(os.environ.get("CES_DVE64", "16"))  # DVE64/64 of sin cols on DVE
POOL8 = int(os.environ.get("CES_POOL8", "4"))  # POOL8/8 of mul cols on Pool
SINCH = int(os.environ.get("CES_SINCH", "2048"))  # ACT sin chunk cols
WARM = int(os.environ.get("CES_WARM", "0"))  # dummy ldweights per pair (HAM)
MAGIC = float(1.5 * 2 ** 23)
# minimax sin(2*pi*f) = f*(c1 + c3 s + c5 s^2 + c7 s^3), s=f^2, f in [-.5,.5]
C1S, C3S, C5S, C7S = 6.2786854, -41.09533605, 77.94322921, -56.11589314
# rescaled so the u^7 coefficient is exactly -1:  u = KS*f,
# sin(2*pi*f) = u*(D1 + D3 u^2 + D5 u^4 - u^6)
KS = float((-C7S) ** (1.0 / 7.0))
D1, D3, D5 = C1S / KS, C3S / KS ** 3, C5S / KS ** 5

_SETUP_DONE = False
ADD_FRAC_B = None
SIN_POLY7 = None


def _mk_op(name, opcode, spec, rd1):
    shas = {}
    for ver in ("v3", "v4"):
        try:
            s = DveOpSpec(name=name, opcode=opcode, uops=lower(spec, ver=ver),
                          rd1_en=rd1)
            shas[ver] = s.sha(ver)
        except Exception:
            pass
    op = dve_ops.DveOp(name, spec, subdim=False, uops_sha=shas)
    dve_ops.OPS.append(op)
    dve_ops._SUB_OPCODE_FOR_NAME[name] = opcode
    dve_ops.CUSTOM_DVE_SPECS[name] = op.spec
    return op


def _setup_framework():
    global _SETUP_DONE, ADD_FRAC_B, SIN_POLY7
    if _SETUP_DONE:
        return
    # out = frac(in0 + bias) * scale ; bias via s1 ([P,1] AP), magic via s0,
    # scale via imm2 (1.0 for the ACT sin path, KS for the DVE poly path)
    _y = Src0 + C1
    ADD_FRAC_B = _mk_op("ADD_FRAC_B", 17, Spec(
        body=(_y - ((_y + C0) - C0)) * C2,
        reference=lambda in0, in1, s0, s1, imm2:
            ((in0 + s1) - ((in0 + s1 + s0) - s0)) * imm2,
    ), rd1=False)
    # in0 = u = KS*f: out = u*(D1 + D3 u^2 + D5 u^4 - u^6) ~= sin(2*pi*f)
    # (no second tensor stream: rd1 custom ops measured ~4.5x slower)
    _u = Src0
    _s = sq(_u)
    SIN_POLY7 = _mk_op("SIN_POLY7", 18, Spec(
        body=_u * (C0 + _s * (C1 + _s * (C2 - _s))),
        reference=lambda in0, in1, s0, s1, imm2:
            in0 * (s0 + (in0 * in0) * (s1 + (in0 * in0)
                   * (imm2 - (in0 * in0)))),
    ), rd1=False)

    _real_tables = hw_specs.get_activation_tables

    def _patched_tables(arch):
        t = _real_tables(arch)
        for _name, s in t.items():
            s.discard(AF.Sin)
        t["exp_and_friends"].add(AF.Sin)
        return t

    bacc.get_activation_tables = _patched_tables
    _SETUP_DONE = True


_NC_CACHE = None


def _build_program():
    global _NC_CACHE
    if _NC_CACHE is not None:
        return _NC_CACHE
    _setup_framework()

    nc = bacc.Bacc()
    xtd = nc.dram_tensor("xt", [128, B_CORE], dt.float16, kind="ExternalInput")
    wcat_d = nc.dram_tensor("wcat", [I, 3 * O], dt.float16,
                            kind="ExternalInput")
    cvec_d = nc.dram_tensor("cvec", [128, 1], dt.float32, kind="ExternalInput")
    outd = nc.dram_tensor("out", [128, B_CORE], dt.float16,
                          kind="ExternalOutput")

    with TileContext(nc) as tc:
        with tc.tile_pool(name="const", bufs=1) as cpool, \
             tc.tile_pool(name="xb", bufs=6) as xb_pool, \
             tc.tile_pool(name="ob", bufs=4) as ob_pool, \
             tc.tile_pool(name="grp", bufs=3) as gpool, \
             tc.tile_pool(name="magps", bufs=2, space="PSUM") as mag_psum, \
             tc.tile_pool(name="phips", bufs=(3 if WARM else 2),
                          space="PSUM") as phi_psum, \
             tc.tile_pool(name="warmps", bufs=1, space="PSUM") as warm_psum:

            # scratch PSUM bank for dep-free PE-warming dummy matmuls (the
            # HAM clock gate re-throttles the PE to 1.2GHz when its duty
            # cycle drops; see engines/01-tensor-engine.md)
            warm_ps = warm_psum.tile([128, MMW], dt.float32,
                                     name="warm_ps") if WARM else None

            wcat = cpool.tile([I, 3 * O], dt.float16)
            nc.sync.dma_start(out=wcat, in_=wcat_d[:, :])
            cvec = cpool.tile([128, 1], dt.float32)
            nc.sync.dma_start(out=cvec, in_=cvec_d[:, :])
            bias_t = cvec[:, 0:1]

            pend = None    # (outs, mag16, sin16, col0, S, NA, pool8) of s-1

            def flush_pool(pend):
                """Pool multiply for super s-1 on the FRONT columns (their
                sin chunk lands earliest), issued at the start of super s's
                emission: it runs while DVE chews on s's fracs, so it never
                overlaps the DVE multiply (concurrent Pool+DVE muls collapse
                the DVE 2x packed mode ~4-15x)."""
                p_outs, p_mag, p_sin, p_col0, S, NA, pl8 = pend
                pc = S * pl8 // 8
                if pc > 0:
                    nc.gpsimd.tensor_mul(p_outs[:, 0:pc], p_mag[:, 0:pc],
                                         p_sin[:, 0:pc])
                    # ship the Pool-multiplied front half immediately; the
                    # back half goes out with flush_vec_dma
                    nc.gpsimd.dma_start(out=outd[:, p_col0:p_col0 + pc],
                                        in_=p_outs[:, 0:pc])

            def flush_vec_dma(pend):
                """DVE multiply for super s-1 on the BACK columns (emitted
                mid-super-s, after the frac pairs — by then s-1's last sin
                chunk and poly are long done and the Pool mul has finished,
                so the binding DVE engine never idles on it) + the output
                DMA, emitted after BOTH muls so the Tile dep tracker orders
                the DMA read behind the DVE writes."""
                p_outs, p_mag, p_sin, p_col0, S, NA, pl8 = pend
                pc = S * pl8 // 8
                if pc < S:
                    nc.vector.tensor_mul(p_outs[:, pc:S], p_mag[:, pc:S],
                                         p_sin[:, pc:S])
                nc.gpsimd.dma_start(out=outd[:, p_col0 + pc:p_col0 + S],
                                    in_=p_outs[:, pc:S])

            col0 = 0
            for s, S in enumerate(SUPERS):
                NA = S - (S * DVE64) // 64
                xs_full = xb_pool.tile([128, 4096], dt.float16, tag="xs",
                                       name="xs")
                xs = xs_full[:, 0:S]
                # split the input DMA so the first pairs' matmuls can start
                # before the whole super has landed (Tile deps are
                # range-aware)
                for d0 in range(0, S, 1024):
                    d1 = min(d0 + 1024, S)
                    nc.sync.dma_start(out=xs[:, d0:d1],
                                      in_=xtd[:, col0 + d0:col0 + d1])

                mag16 = gpool.tile([128, 4096], dt.float16, tag="mag16",
                                   name="mag16")[:, 0:S]
                f16 = gpool.tile([128, 4096], dt.float16, tag="f16",
                                 name="f16")[:, 0:S]
                sin16 = gpool.tile([128, 4096], dt.float16, tag="sin16",
                                   name="sin16")[:, 0:S]
                outs = ob_pool.tile([128, 4096], dt.float16, tag="outs",
                                    name="outs")[:, 0:S]

                if pend is not None:
                    flush_pool(pend)

                sin_done = 0           # f16 cols already sent to ACT sin
                for p0 in range(0, S, PAIR):
                    PW = min(PAIR, S - p0)
                    mag_ps = mag_psum.tile([128, PAIR], dt.float32,
                                           tag="magps", name="mag_ps")[:, 0:PW]
                    nmm = PW // MMW
                    if WARM:
                        phis = [(p0 + h * MMW, p0 + (h + 1) * MMW,
                                 phi_psum.tile([128, MMW], dt.float32,
                                               tag="phips", name="phi_ps"))
                                for h in range(nmm)]
                    else:
                        phi_ps = phi_psum.tile([128, PAIR], dt.float32,
                                               tag="phips",
                                               name="phi_ps")[:, 0:PW]
                        phis = [(p0, p0 + PW, phi_ps)]
                    for h in range(nmm):
                        cc = slice(p0 + h * MMW, p0 + (h + 1) * MMW)
                        pc = slice(h * MMW, (h + 1) * MMW)
                        nc.tensor.matmul(mag_ps[:, pc], wcat[:, 0:128],
                                         xs[:, cc], start=True, stop=True)
                    for lo, hi, ph in phis:
                        for h in range((hi - lo) // MMW):
                            cc = slice(lo + h * MMW, lo + (h + 1) * MMW)
                            pc = slice(h * MMW, (h + 1) * MMW)
                            nc.tensor.matmul(ph[:, pc], wcat[:, 128:256],
                                             xs[:, cc], start=True,
                                             stop=(not USE_TL))
                            if USE_TL:
                                nc.tensor.matmul(ph[:, pc],
                                                 wcat[:, 256:384],
                                                 xs[:, cc], start=False,
                                                 stop=True)
                    # dep-free PE filler: keeps the HAM activity monitor from
                    # re-throttling the PE clock to 1.2GHz during idle gaps
                    for _ in range(WARM):
                        nc.tensor.matmul(warm_ps[:, 0:384], wcat[:, 0:128],
                                         wcat[:, 0:384], start=True,
                                         stop=True)

                    nc.scalar.activation(mag16[:, p0:p0 + PW], mag_ps,
                                         AF.Exp, bias=0.0, scale=1.0)
                    # range-reduce; cols >= NA feed the DVE poly (scale KS)
                    for lo, hi, ph in phis:
                        if lo < NA:
                            e = min(hi, NA)
                            nc.vector._custom_dve(
                                ADD_FRAC_B, out=f16[:, lo:e],
                                in0=ph[:, 0:e - lo], s0=MAGIC, s1=bias_t,
                                imm2=1.0)
                        if hi > NA:
                            b = max(lo, NA)
                            nc.vector._custom_dve(
                                ADD_FRAC_B, out=f16[:, b:hi],
                                in0=ph[:, b - lo:hi - lo], s0=MAGIC,
                                s1=bias_t, imm2=KS)
                    # ACT sin in SINCH-col chunks as soon as deps allow
                    avail = min(p0 + PW, NA)
                    while avail - sin_done >= SINCH or \
                            (avail >= NA and avail > sin_done):
                        c1 = min(sin_done + SINCH, NA)
                        nc.scalar.activation(
                            sin16[:, sin_done:c1], f16[:, sin_done:c1],
                            AF.Sin, bias=0.0, scale=1.0)
                        sin_done = c1

                if pend is not None:
                    flush_vec_dma(pend)

                # DVE poly sin
                if NA < S:
                    nc.vector._custom_dve(
                        SIN_POLY7, out=sin16[:, NA:S],
                        in0=f16[:, NA:S], s0=D1, s1=D3, imm2=D5)

                pend = (outs, mag16, sin16, col0, S, NA,
                        POOL8 if s < len(SUPERS) - 1 else 0)
                col0 += S

            flush_pool(pend)
            flush_vec_dma(pend)

    nc.compile()
    _real_tjb = nc.to_json_bytes
    nc.to_json_bytes = lambda: _real_tjb().replace(b'"func":"Sin"',
                                                   b'"func":"Sin2pi"')
    _NC_CACHE = nc
    return nc


LAST_RESULT = None


def kernel(x, w_real, w_imag, bias):
    global LAST_RESULT
    x = np.asarray(x, dtype=np.float32)
    w_real = np.asarray(w_real, dtype=np.float32)
    w_imag = np.asarray(w_imag, dtype=np.float32)
    bias = np.asarray(bias, dtype=np.float32)

    wr = w_real.astype(np.float64)
    wi = w_imag.astype(np.float64)
    L = 0.5 * np.log(wr * wr + wi * wi)            # [O, I] log|w|
    T = np.arctan2(wi, wr) / (2 * np.pi)           # [O, I] angle in turns
    Lh = np.ascontiguousarray(L.T).astype(np.float16)
    TT = np.ascontiguousarray(T.T)                 # [I, O]
    Th = TT.astype(np.float16)
    Tl = (TT - Th.astype(np.float64)).astype(np.float16)
    wcat = np.concatenate([Lh, Th, Tl], axis=1)    # [I, 3*O] fp16
    bp = ((bias.astype(np.float64) + np.pi / 2) / (2 * np.pi)).astype(
        np.float32)                                # [O] bias in turns + 1/4
    cvec = np.ascontiguousarray(bp[:, None], dtype=np.float32)  # [128, 1]

    xh = x.astype(np.float16)                      # [B, I]

    nc = _build_program()

    in_maps = []
    for c in range(N_CORES):
        xt = np.ascontiguousarray(xh[c * B_CORE:(c + 1) * B_CORE, :].T)
        in_maps.append({"xt": xt, "wcat": wcat, "cvec": cvec})

    trace = bool(int(os.environ.get("CES_TRACE", "0")))
    if trace:
        _install_trace_shim()
    try:
        res = run_bass_kernel_spmd(nc, in_maps, core_ids=list(range(N_CORES)),
                                   trace=trace)
    except Exception:
        import time
        time.sleep(2.0)
        res = run_bass_kernel_spmd(nc, in_maps, core_ids=list(range(N_CORES)),
                                   trace=False)
    LAST_RESULT = res
    if trace and res.exec_time_ns is not None:
        print(f"HW exec time: {res.exec_time_ns} ns", flush=True)

    out = np.empty((B, O), dtype=np.float32)
    for c in range(N_CORES):
        out[c * B_CORE:(c + 1) * B_CORE, :] = \
            res.results[c]["out"].T.astype(np.float32)
    return out


def _install_trace_shim():
    """NTFF profiling hook shim (this image's antenv lacks axon_hooks)."""
    import sys
    import types
    import importlib.util as ilu
    if ilu.find_spec("antenv.axon_hooks") is None and \
            "antenv.axon_hooks" not in sys.modules:
        m = types.ModuleType("antenv.axon_hooks")
        h = [None]
        m.set_axon_ntff_profile_hook = lambda v: h.__setitem__(0, v)
        m.get_axon_ntff_profile_hook = lambda: h[0]
        sys.modules["antenv.axon_hooks"] = m
        sys.path.insert(0, "/root/.axon_site")
        try:
            from trn_agent_boot.trn_boot import _ntff_profile_via_ctypes
            m.set_axon_ntff_profile_hook(
                _ntff_profile_via_ctypes("/opt/axon/libaxon_pjrt.so"))
        except Exception:
            pass
    import concourse.bass_utils as bu
    bu.upload_artifacts = lambda d: "local://skipped"


if __name__ == "__main__":
    rng = np.random.default_rng(0)
    x = rng.uniform(-1, 1, (B, I)).astype(np.float32)
    th = rng.uniform(-np.pi, np.pi, (O, I)).astype(np.float32)
    sc = np.exp(0.1 * rng.standard_normal((O, I))).astype(np.float32)
    wr = sc * np.cos(th)
    wi = sc * np.sin(th)
    bs = rng.uniform(-np.pi, np.pi, (O,)).astype(np.float32)
    got = kernel(x, wr, wi, bs)
    absw = np.sqrt(wr.astype(np.float64)**2 + wi.astype(np.float64)**2)
    angw = np.arctan2(wi.astype(np.float64), wr.astype(np.float64))
    mag = np.exp(x.astype(np.float64) @ np.log(absw).T)
    y = x.astype(np.float64) @ angw.T + bs
    ref = np.cos(y) * mag
    err = np.abs(got - ref)
    print(f"absmax={err.max():.3e} scale={np.abs(ref).max():.2f} "
          f"absmax/scale={err.max()/np.abs(ref).max():.3e} "
          f"relL2={np.linalg.norm(got-ref)/np.linalg.norm(ref):.3e}")

